# revision 43
# baseline (speedup 1.0000x reference)
"""Causal self-attention Trainium2 kernel (v3, bf16).

Full inputs -> full outputs. Data-parallel over batch across 8 NeuronCores
(16 batches per core), no collectives.

Per-core strategy (all matmuls in bf16, fp32 PSUM accumulation):
  - X is transposed + cast to bf16 on the HOST: XT [C, tok] uploaded
    directly (no PE transposes, half the input DMA bytes).
  - Q^T/K^T [feature, tok]: lhsT = w_attn^T tile (Q part pre-scaled by
    1/sqrt(hd) on host), rhs = XT; bias folded into the PSUM->SBUF
    eviction (ACT engine).
  - V [tok, feature] with an interleaved ones column per head (65-wide),
    so row 64 of the PV output is Z = sum_k P.
  - Scores S^T[k, q] in a 384-col PSUM tile (k-chunk0 x q 0:256, then
    k-chunk1 x q 128:256; the fully-masked quarter is never computed).
    Causal mask (-1e30) DMA'd into PSUM ahead of the matmuls, score
    matmuls accumulate on top (start=False).
  - P = exp(S^T) on ACT -> bf16.
  - PV: lhsT = V65 tile, rhs = P -> O^T (+Z row) in PSUM.
  - Normalize: Z -> bf16 SBUF (ACT), K=1 matmul broadcasts Z across 64
    partitions into the same PSUM bank's free half, DVE
    reciprocal_approx_fast on the broadcast, one DVE multiply -> O^T bf16.
  - Projection: O^T head-pair-packed [128, 3, tok] -> 3 K=128 matmuls per
    tile + 1 K=1 ones matmul adding the effective bias; the result is
    DMA'd to DRAM directly from PSUM (no SBUF staging).
"""

import numpy as np

import concourse.bass as bass
import concourse.bacc as bacc
import concourse.mybir as mybir
import concourse.tile as tile

N_CORES = 8
B, T, C = 128, 256, 384
H, HD = 6, 64
NB = B // N_CORES          # batches per core (16)
TOK = NB * T               # tokens per core (4096)
G = 2                      # batches per group
NG = NB // G               # groups per core (8)
GT = G * T                 # tokens per group (512)
NTT = GT // 128            # 128-token tiles per group (4)
F32 = mybir.dt.float32
BF16 = mybir.dt.bfloat16
AF = mybir.ActivationFunctionType
NEGBIG = -1.0e30

def _body(tc, xt_d, wat_d, wpt_d, bqk_d, beff_d, mask_d, ones_d,
          onesH_d, ident_d, y_d):
    nc = tc.nc
    from contextlib import ExitStack

    ctx = ExitStack()
    with ctx:
        const = ctx.enter_context(tc.tile_pool(name="const", bufs=1))
        xt = ctx.enter_context(tc.tile_pool(name="xt", bufs=2))
        qkt = ctx.enter_context(tc.tile_pool(name="qkt", bufs=2))
        v65 = ctx.enter_context(tc.tile_pool(name="v65", bufs=2))
        pp = ctx.enter_context(tc.tile_pool(name="pp", bufs=4))
        zp = ctx.enter_context(tc.tile_pool(name="zp", bufs=4))
        rp = ctx.enter_context(tc.tile_pool(name="rp", bufs=4))
        ot = ctx.enter_context(tc.tile_pool(name="ot", bufs=2))
        yb = ctx.enter_context(tc.tile_pool(name="yb", bufs=4))
        mm_ps = ctx.enter_context(tc.tile_pool(name="mm_ps", bufs=2, space="PSUM"))
        s_ps = ctx.enter_context(tc.tile_pool(name="s_ps", bufs=3, space="PSUM"))
        o_ps = ctx.enter_context(tc.tile_pool(name="o_ps", bufs=3, space="PSUM"))

        # DMA queue split: input loads trigger from the (idle) Pool queue,
        # y stores from the Sync queue — a store waiting on proj can't
        # block the next group's XT prefetch.
        dma_in = nc.gpsimd.dma_start
        dma_out = nc.sync.dma_start

        wat_sb = const.tile([128, 3, 3 * C], BF16, name="wat_sb")
        wpt_sb = const.tile([128, 3, C], BF16, name="wpt_sb")
        bqk_sb = const.tile([128, 6], F32, name="bqk_sb")
        beff_sb = const.tile([128, C], F32, name="beff_sb")
        mask_sb = const.tile([128, 384], BF16, name="mask_sb")
        ones_sb = const.tile([128, 128], BF16, name="ones_sb")
        onesH_sb = const.tile([128, H], BF16, name="onesH_sb")
        ident_sb = const.tile([128, 128], BF16, name="ident_sb")

        xtv = xt_d.ap()                                   # [128, NG, 3, GT]
        yv = y_d.ap().rearrange("(g tt p) c -> g tt p c", tt=NTT, p=128)

        xt_tiles = {}

        def load_xt(g):
            t = xt.tile([128, 3, GT], BF16, name=f"XT_{g}", tag="XT")
            dma_in(t[:], xtv[:, g])
            xt_tiles[g] = t

        # XT on the Pool queue, consts on the Sync queue: the first QKT
        # matmul's two inputs arrive in parallel.
        load_xt(0)
        for ct in range(3):
            dma_out(wat_sb[:, ct, :], wat_d.ap()[:, ct, :])
        dma_out(wpt_sb[:], wpt_d.ap())
        dma_out(bqk_sb[:], bqk_d.ap())
        dma_out(beff_sb[:], beff_d.ap())
        dma_out(mask_sb[:], mask_d.ap())
        dma_out(ones_sb[:], ones_d.ap())
        dma_out(onesH_sb[:], onesH_d.ap())
        dma_out(ident_sb[:], ident_d.ap())

        for g in range(NG):
            XT_sb = xt_tiles.pop(g)

            # ---- Q^T / K^T  [feature, tok] (ft 0..2 = Q chunks, 3..5 = K)
            # K/Q pairs evicted in the order the score chains consume them.
            QKT_sb = qkt.tile([128, 6, GT], BF16, name=f"QKT_{g}", tag="QKT")
            for ft in (3, 0, 4, 1, 5, 2):
                ps = mm_ps.tile([128, 512], F32, name=f"psqk_{g}_{ft}", tag="mm")
                for ct in range(3):
                    nc.tensor.matmul(
                        ps[:],
                        wat_sb[:, ct, 128 * ft:128 * (ft + 1)],
                        XT_sb[:, ct, :],
                        start=(ct == 0),
                        stop=(ct == 2),
                    )
                nc.scalar.activation(QKT_sb[:, ft, :], ps[:], AF.Identity,
                                     bias=bqk_sb[:, ft:ft + 1], scale=1.0)

            # ---- V [tok, feature] with interleaved ones column
            V65_sb = v65.tile([128, NTT, H * 65], BF16, name=f"V65_{g}", tag="V65")
            for tt in range(NTT):
                psv = o_ps.tile([128, 512], F32, name=f"psv_{g}_{tt}", tag="o")
                for ct in range(3):
                    nc.tensor.matmul(
                        psv[:, 0:C],
                        XT_sb[:, ct, 128 * tt:128 * (tt + 1)],
                        wat_sb[:, ct, 2 * C:3 * C],
                        start=(ct == 0),
                        stop=(ct == 2),
                    )
                v_view = V65_sb[:, tt, :].rearrange("p (h w) -> p h w", h=H)
                nc.vector.tensor_copy(
                    v_view[:, :, 0:64],
                    psv[:, 0:C].rearrange("p (h w) -> p h w", h=H),
                )
                nc.gpsimd.tensor_copy(v_view[:, :, 64:65],
                                      onesH_sb[:].unsqueeze(2))
            if g + 1 < NG:
                load_xt(g + 1)

            # ---- attention, software-pipelined over 12 (bl, h) chains
            OT_sb = ot.tile([128, 3, GT], BF16, name=f"OT_{g}", tag="OT")
            chains = [(bl, h) for bl in range(G) for h in range(H)]
            st = {}

            def stage0(i):
                bl, h = chains[i]
                ft, row0, q0 = h // 2, 64 * (h % 2), 256 * bl
                ps_s = s_ps.tile([128, 384], F32, name=f"pss_{g}_{i}", tag="s")
                # mask init rotated across engines to offload the PE
                r = i % 12
                if r < 5:
                    nc.tensor.matmul(ps_s[:], ident_sb[:], mask_sb[:],
                                     start=True, stop=False,
                                     skip_group_check=True)
                elif r < 9:
                    nc.scalar.copy(ps_s[:], mask_sb[:])
                else:
                    nc.vector.tensor_copy(ps_s[:], mask_sb[:])
                KT = QKT_sb[row0:row0 + 64, 3 + ft, :]
                QT = QKT_sb[row0:row0 + 64, ft, :]
                nc.tensor.matmul(
                    ps_s[:, 0:256],
                    KT[:, q0:q0 + 128],
                    QT[:, q0:q0 + 256],
                    start=False, stop=False, skip_group_check=True,
                )
                nc.tensor.matmul(
                    ps_s[:, 256:384],
                    KT[:, q0 + 128:q0 + 256],
                    QT[:, q0 + 128:q0 + 256],
                    start=False, stop=True, skip_group_check=True,
                )
                P_sb = pp.tile([128, 384], BF16, name=f"P_{g}_{i}", tag="P")
                nc.scalar.activation(P_sb[:], ps_s[:], AF.Exp)
                st[i] = (ps_s, P_sb)

            def stage1(i):
                bl, h = chains[i]
                _, P_sb = st[i]
                o_t = o_ps.tile([128, 512], F32, name=f"pso_{g}_{i}", tag="o")
                nc.tensor.matmul(o_t[0:65, 0:256],
                                 V65_sb[:, 2 * bl, 65 * h:65 * h + 65],
                                 P_sb[:, 0:256],
                                 start=True, stop=False, skip_group_check=True)
                nc.tensor.matmul(o_t[0:65, 128:256],
                                 V65_sb[:, 2 * bl + 1, 65 * h:65 * h + 65],
                                 P_sb[:, 256:384],
                                 start=False, stop=True, skip_group_check=True)
                z_sb = zp.tile([128, 256], BF16, name=f"z_{g}_{i}", tag="z")
                nc.scalar.copy(z_sb[64:65, :], o_t[64:65, 0:256])
                st[i] = (o_t, z_sb)

            def stage2(i):
                bl, h = chains[i]
                ft, row0, q0 = h // 2, 64 * (h % 2), 256 * bl
                o_t, z_sb = st.pop(i)
                nc.tensor.matmul(o_t[0:64, 256:512],
                                 ones_sb[64:65, 0:64],
                                 z_sb[64:65, :],
                                 start=True, stop=True, skip_group_check=True)
                rbc_sb = rp.tile([128, 256], F32, name=f"r_{g}_{i}", tag="r")
                nc.vector.reciprocal_approx_fast(rbc_sb[0:64, :],
                                                 o_t[0:64, 256:512])
                nc.vector.tensor_mul(OT_sb[row0:row0 + 64, ft, q0:q0 + 256],
                                     o_t[0:64, 0:256], rbc_sb[0:64, :])

            def proj(tt):
                ps_y = mm_ps.tile([128, 512], F32, name=f"psy_{g}_{tt}", tag="mm")
                for fp in range(3):
                    nc.tensor.matmul(
                        ps_y[:, 0:C],
                        OT_sb[:, fp, 128 * tt:128 * (tt + 1)],
                        wpt_sb[:, fp, :],
                        start=(fp == 0),
                        stop=(fp == 2),
                    )
                Y_sb = yb.tile([128, C], F32, name=f"Y_{g}_{tt}", tag="Y")
                nc.vector.tensor_add(Y_sb[:], ps_y[:, 0:C], beff_sb[:])
                dma_out(yv[g, tt], Y_sb[:])

            n = len(chains)
            for i in range(n + 2):
                if i < n:
                    stage0(i)
                if 1 <= i <= n:
                    stage1(i - 1)
                if 2 <= i <= n + 1:
                    stage2(i - 2)
                if i == 9:
                    # bl=0 chains (idx 0..5) fully normalized after i==7
                    proj(0)
                    proj(1)
            proj(2)
            proj(3)


_CACHE = {}


def _build_nc():
    if "nc" in _CACHE:
        return _CACHE["nc"]
    nc = bacc.Bacc("TRN2", target_bir_lowering=False, debug=False,
                   num_devices=N_CORES)
    xt_d = nc.dram_tensor("xt", [128, NG, 3, GT], BF16, kind="ExternalInput")
    wat_d = nc.dram_tensor("wat", [128, 3, 3 * C], BF16, kind="ExternalInput")
    wpt_d = nc.dram_tensor("wpt", [128, 3, C], BF16, kind="ExternalInput")
    bqk_d = nc.dram_tensor("bqk", [128, 6], F32, kind="ExternalInput")
    beff_d = nc.dram_tensor("beff", [128, C], F32, kind="ExternalInput")
    mask_d = nc.dram_tensor("maskS", [128, 384], BF16, kind="ExternalInput")
    ones_d = nc.dram_tensor("onesb", [128, 128], BF16, kind="ExternalInput")
    onesH_d = nc.dram_tensor("onesH", [128, H], BF16, kind="ExternalInput")
    ident_d = nc.dram_tensor("identb", [128, 128], BF16, kind="ExternalInput")
    y_d = nc.dram_tensor("y", [TOK, C], F32, kind="ExternalOutput")

    with tile.TileContext(nc) as tc:
        _body(tc, xt_d, wat_d, wpt_d, bqk_d, beff_d, mask_d, ones_d,
              onesH_d, ident_d, y_d)
    nc.compile()
    _CACHE["nc"] = nc
    return nc


def _host_inputs(x, w_attn, b_attn, w_proj, b_proj):
    """Per-core input maps (host-side prep: transposes, packing, bf16)."""
    import ml_dtypes

    bf16 = ml_dtypes.bfloat16

    # w_attn^T with Q columns pre-scaled by 1/sqrt(hd)
    w_attnT = np.ascontiguousarray(w_attn.T).astype(np.float32).copy()
    w_attnT[:, :C] *= 0.125
    wat = w_attnT.reshape(3, 128, 3 * C).transpose(1, 0, 2)      # [128,3,1152]

    # proj weights, head-pair-packed rows: wpt[p, fp, n] = w_proj[n, 128*fp+p]
    wpt = w_proj.T.reshape(3, 128, C).transpose(1, 0, 2)         # [128,3,384]

    bq = (0.125 * b_attn[:C]).reshape(3, 128).T                  # [128,3]
    bk = b_attn[C:2 * C].reshape(3, 128).T                       # [128,3]
    bqk = np.concatenate([bq, bk], axis=1)                       # [128,6]

    b_eff = b_proj + w_proj @ b_attn[2 * C:]                     # [384]

    # mask for the 384-col S^T bank: cols j<256: (k=p, q=j);
    # cols j in [256,384): (k=128+p, q=j-128)
    p = np.arange(128)[:, None]
    j = np.arange(384)[None, :]
    valid = np.where(j < 256, p <= j, p <= j - 256)
    mask = np.where(valid, 0.0, NEGBIG)

    common = {
        "wat": np.ascontiguousarray(wat).astype(bf16),
        "wpt": np.ascontiguousarray(wpt).astype(bf16),
        "bqk": np.ascontiguousarray(bqk).astype(np.float32),
        "beff": np.ascontiguousarray(
            np.broadcast_to(b_eff, (128, C))).astype(np.float32),
        "maskS": mask.astype(bf16),
        "onesb": np.ones((128, 128), dtype=bf16),
        "onesH": np.ones((128, H), dtype=bf16),
        "identb": np.eye(128).astype(bf16),
    }
    xs = x.reshape(N_CORES, TOK, C)
    in_maps = []
    for c in range(N_CORES):
        xtc = xs[c].T.reshape(3, 128, NG, GT).transpose(1, 2, 0, 3)
        m = dict(common)
        m["xt"] = np.ascontiguousarray(xtc).astype(bf16)
        in_maps.append(m)
    return in_maps


def kernel(x, w_attn, b_attn, w_proj, b_proj):
    from concourse.bass_utils import run_bass_kernel_spmd

    x = np.asarray(x, dtype=np.float32)
    w_attn = np.asarray(w_attn, dtype=np.float32)
    b_attn = np.asarray(b_attn, dtype=np.float32)
    w_proj = np.asarray(w_proj, dtype=np.float32)
    b_proj = np.asarray(b_proj, dtype=np.float32)

    nc = _build_nc()
    in_maps = _host_inputs(x, w_attn, b_attn, w_proj, b_proj)
    res = run_bass_kernel_spmd(nc, in_maps, core_ids=list(range(N_CORES)))
    y = np.stack([res.results[c]["y"] for c in range(N_CORES)])
    return y.reshape(B, T, C)


# revision 44
# speedup vs baseline: 1.2294x; 1.2294x over previous
"""Causal self-attention Trainium2 kernel (v3, bf16).

Full inputs -> full outputs. Data-parallel over batch across 8 NeuronCores
(16 batches per core), no collectives.

Per-core strategy (all matmuls in bf16, fp32 PSUM accumulation):
  - X is transposed + cast to bf16 on the HOST: XT [C, tok] uploaded
    directly (no PE transposes, half the input DMA bytes).
  - Q^T/K^T [feature, tok]: lhsT = w_attn^T tile (Q part pre-scaled by
    1/sqrt(hd) on host), rhs = XT; bias folded into the PSUM->SBUF
    eviction (ACT engine).
  - V [tok, feature] with an interleaved ones column per head (65-wide),
    so row 64 of the PV output is Z = sum_k P.
  - Scores S^T[k, q] in a 384-col PSUM tile (k-chunk0 x q 0:256, then
    k-chunk1 x q 128:256; the fully-masked quarter is never computed).
    Causal mask (-1e30) DMA'd into PSUM ahead of the matmuls, score
    matmuls accumulate on top (start=False).
  - P = exp(S^T) on ACT -> bf16.
  - PV: lhsT = V65 tile, rhs = P -> O^T (+Z row) in PSUM.
  - Normalize: Z -> bf16 SBUF (ACT), K=1 matmul broadcasts Z across 64
    partitions into the same PSUM bank's free half, DVE
    reciprocal_approx_fast on the broadcast, one DVE multiply -> O^T bf16.
  - Projection: O^T head-pair-packed [128, 3, tok] -> 3 K=128 matmuls per
    tile + 1 K=1 ones matmul adding the effective bias; the result is
    DMA'd to DRAM directly from PSUM (no SBUF staging).
"""

import numpy as np

import concourse.bass as bass
import concourse.bacc as bacc
import concourse.mybir as mybir
import concourse.tile as tile

N_CORES = 8
B, T, C = 128, 256, 384
H, HD = 6, 64
NB = B // N_CORES          # batches per core (16)
TOK = NB * T               # tokens per core (4096)
G = 2                      # batches per group
NG = NB // G               # groups per core (8)
GT = G * T                 # tokens per group (512)
NTT = GT // 128            # 128-token tiles per group (4)
F32 = mybir.dt.float32
BF16 = mybir.dt.bfloat16
AF = mybir.ActivationFunctionType
NEGBIG = -1.0e30

def _body(tc, xt_d, wat_d, wpt_d, bqk_d, beff_d, mask_d, ones_d,
          onesH_d, ident_d, y_d):
    nc = tc.nc
    from contextlib import ExitStack

    ctx = ExitStack()
    with ctx:
        const = ctx.enter_context(tc.tile_pool(name="const", bufs=1))
        xt = ctx.enter_context(tc.tile_pool(name="xt", bufs=2))
        qkt = ctx.enter_context(tc.tile_pool(name="qkt", bufs=2))
        v65 = ctx.enter_context(tc.tile_pool(name="v65", bufs=2))
        pp = ctx.enter_context(tc.tile_pool(name="pp", bufs=4))
        zp = ctx.enter_context(tc.tile_pool(name="zp", bufs=4))
        rp = ctx.enter_context(tc.tile_pool(name="rp", bufs=4))
        ot = ctx.enter_context(tc.tile_pool(name="ot", bufs=2))
        yb = ctx.enter_context(tc.tile_pool(name="yb", bufs=4))
        mm_ps = ctx.enter_context(tc.tile_pool(name="mm_ps", bufs=2, space="PSUM"))
        s_ps = ctx.enter_context(tc.tile_pool(name="s_ps", bufs=3, space="PSUM"))
        o_ps = ctx.enter_context(tc.tile_pool(name="o_ps", bufs=3, space="PSUM"))

        # DMA queue split: input loads trigger from the (idle) Pool queue,
        # y stores from the Sync queue — a store waiting on proj can't
        # block the next group's XT prefetch.
        dma_in = nc.gpsimd.dma_start
        dma_out = nc.sync.dma_start

        wat_sb = const.tile([128, 3, 3 * C], BF16, name="wat_sb")
        wpt_sb = const.tile([128, 3, C], BF16, name="wpt_sb")
        bqk_sb = const.tile([128, 6], F32, name="bqk_sb")
        beff_sb = const.tile([128, C], F32, name="beff_sb")
        mask_sb = const.tile([128, 384], BF16, name="mask_sb")
        ones_sb = const.tile([128, 128], BF16, name="ones_sb")
        onesH_sb = const.tile([128, H], BF16, name="onesH_sb")
        ident_sb = const.tile([128, 128], BF16, name="ident_sb")

        xtv = xt_d.ap()                                   # [128, NG, 3, GT]
        yv = y_d.ap().rearrange("(g tt p) c -> g tt p c", tt=NTT, p=128)

        xt_tiles = {}

        def load_xt(g):
            t = xt.tile([128, 3, GT], BF16, name=f"XT_{g}", tag="XT")
            dma_in(t[:], xtv[:, g])
            xt_tiles[g] = t

        # XT on the Pool queue, consts on the Sync queue: the first QKT
        # matmul's two inputs arrive in parallel.
        load_xt(0)
        for ct in range(3):
            dma_out(wat_sb[:, ct, :], wat_d.ap()[:, ct, :])
        dma_out(wpt_sb[:], wpt_d.ap())
        dma_out(bqk_sb[:], bqk_d.ap())
        dma_out(beff_sb[:], beff_d.ap())
        dma_out(mask_sb[:], mask_d.ap())
        dma_out(ones_sb[:], ones_d.ap())
        dma_out(onesH_sb[:], onesH_d.ap())
        dma_out(ident_sb[:], ident_d.ap())

        for g in range(NG):
            XT_sb = xt_tiles.pop(g)

            # ---- Q^T / K^T  [feature, tok] (ft 0..2 = Q chunks, 3..5 = K)
            # K/Q pairs evicted in the order the score chains consume them.
            QKT_sb = qkt.tile([128, 6, GT], BF16, name=f"QKT_{g}", tag="QKT")
            for ft in (3, 0, 4, 1, 5, 2):
                ps = mm_ps.tile([128, 512], F32, name=f"psqk_{g}_{ft}", tag="mm")
                for ct in range(3):
                    nc.tensor.matmul(
                        ps[:],
                        wat_sb[:, ct, 128 * ft:128 * (ft + 1)],
                        XT_sb[:, ct, :],
                        start=(ct == 0),
                        stop=(ct == 2),
                    )
                nc.scalar.activation(QKT_sb[:, ft, :], ps[:], AF.Identity,
                                     bias=bqk_sb[:, ft:ft + 1], scale=1.0)

            # ---- V [tok, feature] with interleaved ones column
            V65_sb = v65.tile([128, NTT, H * 65], BF16, name=f"V65_{g}", tag="V65")
            for tt in range(NTT):
                psv = o_ps.tile([128, 512], F32, name=f"psv_{g}_{tt}", tag="o")
                for ct in range(3):
                    nc.tensor.matmul(
                        psv[:, 0:C],
                        XT_sb[:, ct, 128 * tt:128 * (tt + 1)],
                        wat_sb[:, ct, 2 * C:3 * C],
                        start=(ct == 0),
                        stop=(ct == 2),
                    )
                v_view = V65_sb[:, tt, :].rearrange("p (h w) -> p h w", h=H)
                nc.vector.tensor_copy(
                    v_view[:, :, 0:64],
                    psv[:, 0:C].rearrange("p (h w) -> p h w", h=H),
                )
                nc.gpsimd.tensor_copy(v_view[:, :, 64:65],
                                      onesH_sb[:].unsqueeze(2))
            if g + 1 < NG:
                load_xt(g + 1)

            # ---- attention, software-pipelined over 12 (bl, h) chains
            OT_sb = ot.tile([128, 3, GT], BF16, name=f"OT_{g}", tag="OT")
            chains = [(bl, h) for bl in range(G) for h in range(H)]
            st = {}

            def stage0(i):
                bl, h = chains[i]
                ft, row0, q0 = h // 2, 64 * (h % 2), 256 * bl
                ps_s = s_ps.tile([128, 384], F32, name=f"pss_{g}_{i}", tag="s")
                nc.tensor.matmul(ps_s[:], ident_sb[:], mask_sb[:],
                                 start=True, stop=False,
                                 skip_group_check=True)
                KT = QKT_sb[row0:row0 + 64, 3 + ft, :]
                QT = QKT_sb[row0:row0 + 64, ft, :]
                nc.tensor.matmul(
                    ps_s[:, 0:256],
                    KT[:, q0:q0 + 128],
                    QT[:, q0:q0 + 256],
                    start=False, stop=False, skip_group_check=True,
                )
                nc.tensor.matmul(
                    ps_s[:, 256:384],
                    KT[:, q0 + 128:q0 + 256],
                    QT[:, q0 + 128:q0 + 256],
                    start=False, stop=True, skip_group_check=True,
                )
                P_sb = pp.tile([128, 384], BF16, name=f"P_{g}_{i}", tag="P")
                nc.scalar.activation(P_sb[:], ps_s[:], AF.Exp)
                st[i] = (ps_s, P_sb)

            def stage1(i):
                bl, h = chains[i]
                _, P_sb = st[i]
                o_t = o_ps.tile([128, 512], F32, name=f"pso_{g}_{i}", tag="o")
                nc.tensor.matmul(o_t[0:65, 0:256],
                                 V65_sb[:, 2 * bl, 65 * h:65 * h + 65],
                                 P_sb[:, 0:256],
                                 start=True, stop=False, skip_group_check=True)
                nc.tensor.matmul(o_t[0:65, 128:256],
                                 V65_sb[:, 2 * bl + 1, 65 * h:65 * h + 65],
                                 P_sb[:, 256:384],
                                 start=False, stop=True, skip_group_check=True)
                z_sb = zp.tile([128, 256], BF16, name=f"z_{g}_{i}", tag="z")
                nc.scalar.copy(z_sb[64:65, :], o_t[64:65, 0:256])
                st[i] = (o_t, z_sb)

            def stage2(i):
                bl, h = chains[i]
                ft, row0, q0 = h // 2, 64 * (h % 2), 256 * bl
                o_t, z_sb = st.pop(i)
                nc.tensor.matmul(o_t[0:64, 256:512],
                                 ones_sb[64:65, 0:64],
                                 z_sb[64:65, :],
                                 start=True, stop=True, skip_group_check=True)
                rbc_sb = rp.tile([128, 256], F32, name=f"r_{g}_{i}", tag="r")
                nc.vector.reciprocal_approx_fast(rbc_sb[0:64, :],
                                                 o_t[0:64, 256:512])
                nc.vector.tensor_mul(OT_sb[row0:row0 + 64, ft, q0:q0 + 256],
                                     o_t[0:64, 0:256], rbc_sb[0:64, :])

            def proj(tt):
                ps_y = mm_ps.tile([128, 512], F32, name=f"psy_{g}_{tt}", tag="mm")
                for fp in range(3):
                    nc.tensor.matmul(
                        ps_y[:, 0:C],
                        OT_sb[:, fp, 128 * tt:128 * (tt + 1)],
                        wpt_sb[:, fp, :],
                        start=(fp == 0),
                        stop=(fp == 2),
                    )
                Y_sb = yb.tile([128, C], F32, name=f"Y_{g}_{tt}", tag="Y")
                nc.vector.tensor_add(Y_sb[:], ps_y[:, 0:C], beff_sb[:])
                dma_out(yv[g, tt], Y_sb[:])

            n = len(chains)
            for i in range(n + 2):
                if i < n:
                    stage0(i)
                if 1 <= i <= n:
                    stage1(i - 1)
                if 2 <= i <= n + 1:
                    stage2(i - 2)
                if i == 9:
                    # bl=0 chains (idx 0..5) fully normalized after i==7
                    proj(0)
                    proj(1)
            proj(2)
            proj(3)


_CACHE = {}


def _build_nc():
    if "nc" in _CACHE:
        return _CACHE["nc"]
    nc = bacc.Bacc("TRN2", target_bir_lowering=False, debug=False,
                   num_devices=N_CORES)
    xt_d = nc.dram_tensor("xt", [128, NG, 3, GT], BF16, kind="ExternalInput")
    wat_d = nc.dram_tensor("wat", [128, 3, 3 * C], BF16, kind="ExternalInput")
    wpt_d = nc.dram_tensor("wpt", [128, 3, C], BF16, kind="ExternalInput")
    bqk_d = nc.dram_tensor("bqk", [128, 6], F32, kind="ExternalInput")
    beff_d = nc.dram_tensor("beff", [128, C], F32, kind="ExternalInput")
    mask_d = nc.dram_tensor("maskS", [128, 384], BF16, kind="ExternalInput")
    ones_d = nc.dram_tensor("onesb", [128, 128], BF16, kind="ExternalInput")
    onesH_d = nc.dram_tensor("onesH", [128, H], BF16, kind="ExternalInput")
    ident_d = nc.dram_tensor("identb", [128, 128], BF16, kind="ExternalInput")
    y_d = nc.dram_tensor("y", [TOK, C], F32, kind="ExternalOutput")

    with tile.TileContext(nc) as tc:
        _body(tc, xt_d, wat_d, wpt_d, bqk_d, beff_d, mask_d, ones_d,
              onesH_d, ident_d, y_d)
    nc.compile()
    _CACHE["nc"] = nc
    return nc


def _host_inputs(x, w_attn, b_attn, w_proj, b_proj):
    """Per-core input maps (host-side prep: transposes, packing, bf16)."""
    import ml_dtypes

    bf16 = ml_dtypes.bfloat16

    # w_attn^T with Q columns pre-scaled by 1/sqrt(hd)
    w_attnT = np.ascontiguousarray(w_attn.T).astype(np.float32).copy()
    w_attnT[:, :C] *= 0.125
    wat = w_attnT.reshape(3, 128, 3 * C).transpose(1, 0, 2)      # [128,3,1152]

    # proj weights, head-pair-packed rows: wpt[p, fp, n] = w_proj[n, 128*fp+p]
    wpt = w_proj.T.reshape(3, 128, C).transpose(1, 0, 2)         # [128,3,384]

    bq = (0.125 * b_attn[:C]).reshape(3, 128).T                  # [128,3]
    bk = b_attn[C:2 * C].reshape(3, 128).T                       # [128,3]
    bqk = np.concatenate([bq, bk], axis=1)                       # [128,6]

    b_eff = b_proj + w_proj @ b_attn[2 * C:]                     # [384]

    # mask for the 384-col S^T bank: cols j<256: (k=p, q=j);
    # cols j in [256,384): (k=128+p, q=j-128)
    p = np.arange(128)[:, None]
    j = np.arange(384)[None, :]
    valid = np.where(j < 256, p <= j, p <= j - 256)
    mask = np.where(valid, 0.0, NEGBIG)

    common = {
        "wat": np.ascontiguousarray(wat).astype(bf16),
        "wpt": np.ascontiguousarray(wpt).astype(bf16),
        "bqk": np.ascontiguousarray(bqk).astype(np.float32),
        "beff": np.ascontiguousarray(
            np.broadcast_to(b_eff, (128, C))).astype(np.float32),
        "maskS": mask.astype(bf16),
        "onesb": np.ones((128, 128), dtype=bf16),
        "onesH": np.ones((128, H), dtype=bf16),
        "identb": np.eye(128).astype(bf16),
    }
    xs = x.reshape(N_CORES, TOK, C)
    in_maps = []
    for c in range(N_CORES):
        xtc = xs[c].T.reshape(3, 128, NG, GT).transpose(1, 2, 0, 3)
        m = dict(common)
        m["xt"] = np.ascontiguousarray(xtc).astype(bf16)
        in_maps.append(m)
    return in_maps


def kernel(x, w_attn, b_attn, w_proj, b_proj):
    from concourse.bass_utils import run_bass_kernel_spmd

    x = np.asarray(x, dtype=np.float32)
    w_attn = np.asarray(w_attn, dtype=np.float32)
    b_attn = np.asarray(b_attn, dtype=np.float32)
    w_proj = np.asarray(w_proj, dtype=np.float32)
    b_proj = np.asarray(b_proj, dtype=np.float32)

    nc = _build_nc()
    in_maps = _host_inputs(x, w_attn, b_attn, w_proj, b_proj)
    res = run_bass_kernel_spmd(nc, in_maps, core_ids=list(range(N_CORES)))
    y = np.stack([res.results[c]["y"] for c in range(N_CORES)])
    return y.reshape(B, T, C)


# revision 46
# speedup vs baseline: 1.2887x; 1.0482x over previous
"""Causal self-attention Trainium2 kernel (v3, bf16).

Full inputs -> full outputs. Data-parallel over batch across 8 NeuronCores
(16 batches per core), no collectives.

Per-core strategy (all matmuls in bf16, fp32 PSUM accumulation):
  - X is transposed + cast to bf16 on the HOST: XT [C, tok] uploaded
    directly (no PE transposes, half the input DMA bytes).
  - Q^T/K^T [feature, tok]: lhsT = w_attn^T tile (Q part pre-scaled by
    1/sqrt(hd) on host), rhs = XT; bias folded into the PSUM->SBUF
    eviction (ACT engine).
  - V [tok, feature] with an interleaved ones column per head (65-wide),
    so row 64 of the PV output is Z = sum_k P.
  - Scores S^T[k, q] in a 384-col PSUM tile (k-chunk0 x q 0:256, then
    k-chunk1 x q 128:256; the fully-masked quarter is never computed).
    Causal mask (-1e30) DMA'd into PSUM ahead of the matmuls, score
    matmuls accumulate on top (start=False).
  - P = exp(S^T) on ACT -> bf16.
  - PV: lhsT = V65 tile, rhs = P -> O^T (+Z row) in PSUM.
  - Normalize: Z -> bf16 SBUF (ACT), K=1 matmul broadcasts Z across 64
    partitions into the same PSUM bank's free half, DVE
    reciprocal_approx_fast on the broadcast, one DVE multiply -> O^T bf16.
  - Projection: O^T head-pair-packed [128, 3, tok] -> 3 K=128 matmuls per
    tile + 1 K=1 ones matmul adding the effective bias; the result is
    DMA'd to DRAM directly from PSUM (no SBUF staging).
"""

import numpy as np

import concourse.bass as bass
import concourse.bacc as bacc
import concourse.mybir as mybir
import concourse.tile as tile

N_CORES = 8
B, T, C = 128, 256, 384
H, HD = 6, 64
NB = B // N_CORES          # batches per core (16)
TOK = NB * T               # tokens per core (4096)
G = 2                      # batches per group
NG = NB // G               # groups per core (8)
GT = G * T                 # tokens per group (512)
NTT = GT // 128            # 128-token tiles per group (4)
F32 = mybir.dt.float32
BF16 = mybir.dt.bfloat16
AF = mybir.ActivationFunctionType
NEGBIG = -1.0e30

def _body(tc, xt_d, wat_d, wpt_d, bqk_d, beff_d, mask_d, ones_d,
          onesH_d, ident_d, y_d):
    nc = tc.nc
    from contextlib import ExitStack

    ctx = ExitStack()
    with ctx:
        const = ctx.enter_context(tc.tile_pool(name="const", bufs=1))
        xt = ctx.enter_context(tc.tile_pool(name="xt", bufs=2))
        qkt = ctx.enter_context(tc.tile_pool(name="qkt", bufs=2))
        v65 = ctx.enter_context(tc.tile_pool(name="v65", bufs=2))
        pp = ctx.enter_context(tc.tile_pool(name="pp", bufs=4))
        zp = ctx.enter_context(tc.tile_pool(name="zp", bufs=4))
        rp = ctx.enter_context(tc.tile_pool(name="rp", bufs=4))
        ot = ctx.enter_context(tc.tile_pool(name="ot", bufs=2))
        yb = ctx.enter_context(tc.tile_pool(name="yb", bufs=4))
        mm_ps = ctx.enter_context(tc.tile_pool(name="mm_ps", bufs=2, space="PSUM"))
        s_ps = ctx.enter_context(tc.tile_pool(name="s_ps", bufs=3, space="PSUM"))
        o_ps = ctx.enter_context(tc.tile_pool(name="o_ps", bufs=3, space="PSUM"))

        # DMA queue split: input loads trigger from the (idle) Pool queue,
        # y stores from the Sync queue — a store waiting on proj can't
        # block the next group's XT prefetch.
        dma_in = nc.gpsimd.dma_start
        dma_out = nc.sync.dma_start

        wat_sb = const.tile([128, 3, 3 * C], BF16, name="wat_sb")
        wpt_sb = const.tile([128, 3, C], BF16, name="wpt_sb")
        bqk_sb = const.tile([128, 6], F32, name="bqk_sb")
        beff_sb = const.tile([128, C], F32, name="beff_sb")
        mask_sb = const.tile([128, 384], BF16, name="mask_sb")
        ones_sb = const.tile([128, 128], BF16, name="ones_sb")
        onesH_sb = const.tile([128, H], BF16, name="onesH_sb")
        ident_sb = const.tile([128, 128], BF16, name="ident_sb")

        xtv = xt_d.ap()                                   # [128, NG, 3, GT]
        yv = y_d.ap().rearrange("(g tt p) c -> g tt p c", tt=NTT, p=128)

        xt_tiles = {}

        def load_xt(g):
            t = xt.tile([128, 3, GT], BF16, name=f"XT_{g}", tag="XT")
            dma_in(t[:], xtv[:, g])
            xt_tiles[g] = t

        # XT on the Pool queue, consts on the Sync queue: the first QKT
        # matmul's two inputs arrive in parallel.
        load_xt(0)
        for ct in range(3):
            dma_out(wat_sb[:, ct, :], wat_d.ap()[:, ct, :])
        dma_out(wpt_sb[:], wpt_d.ap())
        dma_out(bqk_sb[:], bqk_d.ap())
        dma_out(beff_sb[:], beff_d.ap())
        dma_out(mask_sb[:], mask_d.ap())
        dma_out(ones_sb[:], ones_d.ap())
        dma_out(onesH_sb[:], onesH_d.ap())
        dma_out(ident_sb[:], ident_d.ap())

        for g in range(NG):
            XT_sb = xt_tiles.pop(g)

            # ---- Q^T / K^T  [feature, tok] (ft 0..2 = Q chunks, 3..5 = K)
            # K/Q pairs evicted in the order the score chains consume them.
            QKT_sb = qkt.tile([128, 6, GT], BF16, name=f"QKT_{g}", tag="QKT")
            for ft in (3, 0, 4, 1, 5, 2):
                ps = mm_ps.tile([128, 512], F32, name=f"psqk_{g}_{ft}", tag="mm")
                for ct in range(3):
                    nc.tensor.matmul(
                        ps[:],
                        wat_sb[:, ct, 128 * ft:128 * (ft + 1)],
                        XT_sb[:, ct, :],
                        start=(ct == 0),
                        stop=(ct == 2),
                    )
                nc.scalar.activation(QKT_sb[:, ft, :], ps[:], AF.Identity,
                                     bias=bqk_sb[:, ft:ft + 1], scale=1.0)

            # ---- V [tok, feature] with interleaved ones column.
            # tt 0/1 (needed by bl=0 chains) up front; tt 2/3 deferred into
            # the chain pipeline so their PSUM slots don't wait on the
            # previous group's attention drain.
            V65_sb = v65.tile([128, NTT, H * 65], BF16, name=f"V65_{g}", tag="V65")

            def vtile(tt):
                psv = o_ps.tile([128, 512], F32, name=f"psv_{g}_{tt}", tag="o")
                for ct in range(3):
                    nc.tensor.matmul(
                        psv[:, 0:C],
                        XT_sb[:, ct, 128 * tt:128 * (tt + 1)],
                        wat_sb[:, ct, 2 * C:3 * C],
                        start=(ct == 0),
                        stop=(ct == 2),
                    )
                v_view = V65_sb[:, tt, :].rearrange("p (h w) -> p h w", h=H)
                nc.vector.tensor_copy(
                    v_view[:, :, 0:64],
                    psv[:, 0:C].rearrange("p (h w) -> p h w", h=H),
                )
                nc.gpsimd.tensor_copy(v_view[:, :, 64:65],
                                      onesH_sb[:].unsqueeze(2))

            vtile(0)
            vtile(1)
            if g + 1 < NG:
                load_xt(g + 1)

            # ---- attention, software-pipelined over 12 (bl, h) chains
            OT_sb = ot.tile([128, 3, GT], BF16, name=f"OT_{g}", tag="OT")
            chains = [(bl, h) for bl in range(G) for h in range(H)]
            st = {}

            def stage0(i):
                bl, h = chains[i]
                ft, row0, q0 = h // 2, 64 * (h % 2), 256 * bl
                ps_s = s_ps.tile([128, 384], F32, name=f"pss_{g}_{i}", tag="s")
                nc.tensor.matmul(ps_s[:], ident_sb[:], mask_sb[:],
                                 start=True, stop=False,
                                 skip_group_check=True)
                KT = QKT_sb[row0:row0 + 64, 3 + ft, :]
                QT = QKT_sb[row0:row0 + 64, ft, :]
                nc.tensor.matmul(
                    ps_s[:, 0:256],
                    KT[:, q0:q0 + 128],
                    QT[:, q0:q0 + 256],
                    start=False, stop=False, skip_group_check=True,
                )
                nc.tensor.matmul(
                    ps_s[:, 256:384],
                    KT[:, q0 + 128:q0 + 256],
                    QT[:, q0 + 128:q0 + 256],
                    start=False, stop=True, skip_group_check=True,
                )
                P_sb = pp.tile([128, 384], BF16, name=f"P_{g}_{i}", tag="P")
                nc.scalar.activation(P_sb[:], ps_s[:], AF.Exp)
                st[i] = (ps_s, P_sb)

            def stage1(i):
                bl, h = chains[i]
                _, P_sb = st[i]
                o_t = o_ps.tile([128, 512], F32, name=f"pso_{g}_{i}", tag="o")
                nc.tensor.matmul(o_t[0:65, 0:256],
                                 V65_sb[:, 2 * bl, 65 * h:65 * h + 65],
                                 P_sb[:, 0:256],
                                 start=True, stop=False, skip_group_check=True)
                nc.tensor.matmul(o_t[0:65, 128:256],
                                 V65_sb[:, 2 * bl + 1, 65 * h:65 * h + 65],
                                 P_sb[:, 256:384],
                                 start=False, stop=True, skip_group_check=True)
                z_sb = zp.tile([128, 256], BF16, name=f"z_{g}_{i}", tag="z")
                nc.scalar.copy(z_sb[64:65, :], o_t[64:65, 0:256])
                st[i] = (o_t, z_sb)

            def stage2(i):
                bl, h = chains[i]
                ft, row0, q0 = h // 2, 64 * (h % 2), 256 * bl
                o_t, z_sb = st.pop(i)
                nc.tensor.matmul(o_t[0:64, 256:512],
                                 ones_sb[64:65, 0:64],
                                 z_sb[64:65, :],
                                 start=True, stop=True, skip_group_check=True)
                rbc_sb = rp.tile([128, 256], F32, name=f"r_{g}_{i}", tag="r")
                nc.vector.reciprocal_approx_fast(rbc_sb[0:64, :],
                                                 o_t[0:64, 256:512])
                nc.vector.tensor_mul(OT_sb[row0:row0 + 64, ft, q0:q0 + 256],
                                     o_t[0:64, 0:256], rbc_sb[0:64, :])

            def proj(tt):
                ps_y = mm_ps.tile([128, 512], F32, name=f"psy_{g}_{tt}", tag="mm")
                for fp in range(3):
                    nc.tensor.matmul(
                        ps_y[:, 0:C],
                        OT_sb[:, fp, 128 * tt:128 * (tt + 1)],
                        wpt_sb[:, fp, :],
                        start=(fp == 0),
                        stop=(fp == 2),
                    )
                Y_sb = yb.tile([128, C], F32, name=f"Y_{g}_{tt}", tag="Y")
                nc.vector.tensor_add(Y_sb[:], ps_y[:, 0:C], beff_sb[:])
                dma_out(yv[g, tt], Y_sb[:])

            n = len(chains)
            for i in range(n + 2):
                if i < n:
                    stage0(i)
                if i == 1:
                    vtile(2)
                if i == 2:
                    vtile(3)
                if 1 <= i <= n:
                    stage1(i - 1)
                if 2 <= i <= n + 1:
                    stage2(i - 2)
                if i == 9:
                    # bl=0 chains (idx 0..5) fully normalized after i==7
                    proj(0)
                    proj(1)
            proj(2)
            proj(3)


_CACHE = {}


def _build_nc():
    if "nc" in _CACHE:
        return _CACHE["nc"]
    nc = bacc.Bacc("TRN2", target_bir_lowering=False, debug=False,
                   num_devices=N_CORES)
    xt_d = nc.dram_tensor("xt", [128, NG, 3, GT], BF16, kind="ExternalInput")
    wat_d = nc.dram_tensor("wat", [128, 3, 3 * C], BF16, kind="ExternalInput")
    wpt_d = nc.dram_tensor("wpt", [128, 3, C], BF16, kind="ExternalInput")
    bqk_d = nc.dram_tensor("bqk", [128, 6], F32, kind="ExternalInput")
    beff_d = nc.dram_tensor("beff", [128, C], F32, kind="ExternalInput")
    mask_d = nc.dram_tensor("maskS", [128, 384], BF16, kind="ExternalInput")
    ones_d = nc.dram_tensor("onesb", [128, 128], BF16, kind="ExternalInput")
    onesH_d = nc.dram_tensor("onesH", [128, H], BF16, kind="ExternalInput")
    ident_d = nc.dram_tensor("identb", [128, 128], BF16, kind="ExternalInput")
    y_d = nc.dram_tensor("y", [TOK, C], F32, kind="ExternalOutput")

    with tile.TileContext(nc) as tc:
        _body(tc, xt_d, wat_d, wpt_d, bqk_d, beff_d, mask_d, ones_d,
              onesH_d, ident_d, y_d)
    nc.compile()
    _CACHE["nc"] = nc
    return nc


def _host_inputs(x, w_attn, b_attn, w_proj, b_proj):
    """Per-core input maps (host-side prep: transposes, packing, bf16)."""
    import ml_dtypes

    bf16 = ml_dtypes.bfloat16

    # w_attn^T with Q columns pre-scaled by 1/sqrt(hd)
    w_attnT = np.ascontiguousarray(w_attn.T).astype(np.float32).copy()
    w_attnT[:, :C] *= 0.125
    wat = w_attnT.reshape(3, 128, 3 * C).transpose(1, 0, 2)      # [128,3,1152]

    # proj weights, head-pair-packed rows: wpt[p, fp, n] = w_proj[n, 128*fp+p]
    wpt = w_proj.T.reshape(3, 128, C).transpose(1, 0, 2)         # [128,3,384]

    bq = (0.125 * b_attn[:C]).reshape(3, 128).T                  # [128,3]
    bk = b_attn[C:2 * C].reshape(3, 128).T                       # [128,3]
    bqk = np.concatenate([bq, bk], axis=1)                       # [128,6]

    b_eff = b_proj + w_proj @ b_attn[2 * C:]                     # [384]

    # mask for the 384-col S^T bank: cols j<256: (k=p, q=j);
    # cols j in [256,384): (k=128+p, q=j-128)
    p = np.arange(128)[:, None]
    j = np.arange(384)[None, :]
    valid = np.where(j < 256, p <= j, p <= j - 256)
    mask = np.where(valid, 0.0, NEGBIG)

    common = {
        "wat": np.ascontiguousarray(wat).astype(bf16),
        "wpt": np.ascontiguousarray(wpt).astype(bf16),
        "bqk": np.ascontiguousarray(bqk).astype(np.float32),
        "beff": np.ascontiguousarray(
            np.broadcast_to(b_eff, (128, C))).astype(np.float32),
        "maskS": mask.astype(bf16),
        "onesb": np.ones((128, 128), dtype=bf16),
        "onesH": np.ones((128, H), dtype=bf16),
        "identb": np.eye(128).astype(bf16),
    }
    xs = x.reshape(N_CORES, TOK, C)
    in_maps = []
    for c in range(N_CORES):
        xtc = xs[c].T.reshape(3, 128, NG, GT).transpose(1, 2, 0, 3)
        m = dict(common)
        m["xt"] = np.ascontiguousarray(xtc).astype(bf16)
        in_maps.append(m)
    return in_maps


def kernel(x, w_attn, b_attn, w_proj, b_proj):
    from concourse.bass_utils import run_bass_kernel_spmd

    x = np.asarray(x, dtype=np.float32)
    w_attn = np.asarray(w_attn, dtype=np.float32)
    b_attn = np.asarray(b_attn, dtype=np.float32)
    w_proj = np.asarray(w_proj, dtype=np.float32)
    b_proj = np.asarray(b_proj, dtype=np.float32)

    nc = _build_nc()
    in_maps = _host_inputs(x, w_attn, b_attn, w_proj, b_proj)
    res = run_bass_kernel_spmd(nc, in_maps, core_ids=list(range(N_CORES)))
    y = np.stack([res.results[c]["y"] for c in range(N_CORES)])
    return y.reshape(B, T, C)
